# revision 18
# baseline (speedup 1.0000x reference)
"""v6 experiment: PE transpose-accumulate stream (no folds).

Stream split: DVE 47 slices into an SBUF accumulator; PE transpose-
accumulates 16 slices directly into a TRANSPOSED PSUM accumulator
(2 bank-sized [128,512] f32r tiles, one accumulation group per bank:
first tr start=True, middles start=False/stop=False, the tail closing
trs stop=True). GpSimd only does tail chunk-adds (SBUF). The tail then
has NO folds: final sT[c] = sPE_T[c] (+ T(s_dve[c]) accumulated in the
same group).
"""

import sys

for _p in ("/opt/trn_rl_repo",):
    if _p not in sys.path:
        sys.path.insert(0, _p)

import numpy as np

import concourse.bacc as bacc
import concourse.mybir as mybir
import concourse.tile as tile
from concourse.masks import make_identity
from concourse.bass_utils import run_bass_kernel_spmd

B, N, D = 8192, 64, 128
M = 8
BL = B // M
R = BL // 128
F = BL
H = F // 2
F32 = mybir.dt.float32
F32R = mybir.dt.float32r
NEG_SLOPE = 0.01
LRELU = mybir.ActivationFunctionType.Lrelu

# D -> DVE slice-add, P -> PE transpose-accumulate (8 chunk transposes).
PLAN = (
    [(4, "DDPD"), (4, "DDPD")] * 7
    + [(4, "DPDD"), (2, "DD"), (1, "D")]
)
assert sum(g for g, _ in PLAN) == N - 1
assert sum(r.count("P") for _, r in PLAN) == 15

CHUNK_DMA_ORDER = [4, 5, 6, 7, 0, 1, 2, 3]


def build(load_bufs: int = 7):
    nc = bacc.Bacc(
        "TRN2",
        target_bir_lowering=False,
        debug=False,
        enable_asserts=False,
        num_devices=M,
    )
    e1 = nc.dram_tensor("embedding1", [BL, D], F32, kind="ExternalInput").ap()
    e2 = nc.dram_tensor("all_embeddings2", [N, BL, D], F32, kind="ExternalInput").ap()
    w1 = nc.dram_tensor("W1", [D, D], F32, kind="ExternalInput").ap()
    w2 = nc.dram_tensor("W2", [D, D], F32, kind="ExternalInput").ap()
    out = nc.dram_tensor("out", [D, BL], F32, kind="ExternalOutput").ap()

    e1_r = e1.rearrange("(p r) d -> p (r d)", p=128)
    e2_r = e2.rearrange("n (p r) d -> p n (r d)", p=128)

    with tile.TileContext(nc) as tc:
        with (
            tc.tile_pool(name="const", bufs=1) as cpool,
            tc.tile_pool(name="load", bufs=load_bufs) as lpool,
            tc.tile_pool(name="last", bufs=8) as lastpool,
            tc.tile_pool(name="act", bufs=1) as apool,
            tc.tile_pool(name="spet", bufs=1, space="PSUM") as spool,
            tc.tile_pool(name="ops", bufs=1, space="PSUM") as opool,
            tc.tile_pool(name="trps", bufs=2, space="PSUM") as trpool,
        ):
            ident = cpool.tile([128, 128], F32)
            make_identity(nc, ident[:])
            ident_r = cpool.tile([128, 128], F32R)
            nc.scalar.copy(out=ident_r[:], in_=ident[:])

            w1_sb = cpool.tile([128, 128], F32)
            nc.scalar.dma_start(out=w1_sb[:], in_=w1)
            w2_sb = cpool.tile([128, 128], F32)
            nc.scalar.dma_start(out=w2_sb[:], in_=w2)
            e1_sb = apool.tile([128, F], F32)
            nc.scalar.dma_start(out=e1_sb[:], in_=e1_r)

            w1t_ps = trpool.tile([128, 128], F32, tag="tr")
            nc.tensor.transpose(w1t_ps[:], w1_sb[:], ident[:])
            w1t = cpool.tile([128, 128], F32)
            nc.scalar.copy(out=w1t[:], in_=w1t_ps[:])
            w1t_r = cpool.tile([128, 128], F32R)
            nc.scalar.copy(out=w1t_r[:], in_=w1t_ps[:])
            w2t_ps = trpool.tile([128, 128], F32, tag="tr")
            nc.tensor.transpose(w2t_ps[:], w2_sb[:], ident[:])
            w2t_r = cpool.tile([128, 128], F32R)
            nc.scalar.copy(out=w2t_r[:], in_=w2t_ps[:])

            # Dummy lrelu at the head so its table is resident for the tail.
            warm = cpool.tile([128, 8], F32)
            nc.scalar.activation(warm[:], ident[:, 0:8], LRELU, alpha=NEG_SLOPE)

            e1t = apool.tile([128, F], F32)
            for j in range(R):
                sl = slice(j * 128, (j + 1) * 128)
                tp = trpool.tile([128, 128], F32, tag="tr")
                nc.tensor.transpose(tp[:], e1_sb[:, sl], ident[:])
                nc.scalar.copy(out=e1t[:, sl], in_=tp[:])

            o_ps0 = opool.tile([128, H], F32)
            o_ps1 = opool.tile([128, H], F32)
            o_ps = [o_ps0, o_ps1]
            for h in range(2):
                hs = slice(h * H, (h + 1) * H)
                nc.tensor.matmul(
                    o_ps[h][:], lhsT=w1t[:], rhs=e1t[:, hs], start=True, stop=True
                )

            # Transposed PSUM accumulators: bank A = chunks 0-3, B = 4-7.
            spe_a = spool.tile([128, 512], F32, tag="speA")
            spe_b = spool.tile([128, 512], F32, tag="speB")
            spe = [spe_a, spe_b]
            started = [False, False]

            def tr_acc(src_chunk_ap, c, stop):
                bank = c // 4
                sub = slice((c % 4) * 128, (c % 4 + 1) * 128)
                nc.tensor.matmul(
                    spe[bank][:, sub],
                    lhsT=src_chunk_ap,
                    rhs=ident[:],
                    is_transpose=True,
                    start=not started[bank],
                    stop=stop,
                    skip_group_check=True,
                )
                started[bank] = True

            # ---- stream ----
            s_dve = apool.tile([128, F], F32)
            seen_d = 0
            base = 0
            for gl, routing in PLAN:
                t = lpool.tile([128, gl * F], F32, tag="load")
                nc.sync.dma_start(
                    out=t[:].rearrange("p (n f) -> p n f", n=gl),
                    in_=e2_r[:, base : base + gl, :],
                )
                for g in range(gl):
                    sl = t[:, g * F : (g + 1) * F]
                    if routing[g] == "D":
                        seen_d += 1
                        if seen_d == 1:
                            nc.vector.tensor_copy(out=s_dve[:], in_=sl)
                        else:
                            nc.vector.tensor_add(out=s_dve[:], in0=s_dve[:], in1=sl)
                    else:
                        for c in range(R):
                            tr_acc(
                                t[:, g * F + c * 128 : g * F + (c + 1) * 128],
                                c,
                                False,
                            )
                base += gl

            # Last slice: 8 per-chunk DMAs; DVE adds 4-7, GpSimd 0-3 (SBUF).
            last_t = {}
            for c in CHUNK_DMA_ORDER:
                tcch = lastpool.tile([128, 128], F32, tag=f"lc{c}")
                nc.sync.dma_start(
                    out=tcch[:], in_=e2_r[:, N - 1, c * 128 : (c + 1) * 128]
                )
                last_t[c] = tcch
            for c in [4, 5, 6, 7]:
                sl = slice(c * 128, (c + 1) * 128)
                nc.vector.tensor_add(
                    out=s_dve[:, sl], in0=s_dve[:, sl], in1=last_t[c][:]
                )
            for c in [0, 1, 2, 3]:
                sl = slice(c * 128, (c + 1) * 128)
                nc.gpsimd.tensor_add(
                    out=s_dve[:, sl], in0=s_dve[:, sl], in1=last_t[c][:]
                )

            # ---- tail: closing transposes accumulate T(s_dve[c]); the
            # result IS sT[c] in PSUM. No folds.
            st = apool.tile([128, F], F32R)
            x2t = apool.tile([128, F], F32R)
            out_sb = apool.tile([128, F], F32)

            def tp_of(c):
                bank = c // 4
                sub = slice((c % 4) * 128, (c % 4 + 1) * 128)
                return spe[bank][:, sub]

            for c in [4, 5, 6, 7, 0, 1, 2, 3]:
                sl = slice(c * 128, (c + 1) * 128)
                tr_acc(s_dve[:, sl], c, stop=(c % 4 == 3))

            for c in [4, 5, 6, 7, 0, 1, 2, 3]:
                sl = slice(c * 128, (c + 1) * 128)
                nc.vector.tensor_mul(out=x2t[:, sl], in0=e1t[:, sl], in1=tp_of(c))
            for c in [4, 5, 6, 7, 0, 1, 2, 3]:
                sl = slice(c * 128, (c + 1) * 128)
                nc.scalar.copy(out=st[:, sl], in_=tp_of(c))

            for h in (1, 0):
                hs = slice(h * H, (h + 1) * H)
                nc.tensor.matmul(
                    o_ps[h][:], lhsT=w1t_r[:], rhs=st[:, hs], start=False, stop=False
                )
                nc.tensor.matmul(
                    o_ps[h][:], lhsT=w2t_r[:], rhs=x2t[:, hs], start=False, stop=True
                )
                nc.scalar.activation(out_sb[:, hs], o_ps[h][:], LRELU, alpha=NEG_SLOPE)
                nc.sync.dma_start(out=out[:, hs], in_=out_sb[:, hs])

    nc.compile()
    return nc


_NC = None


def _get_nc():
    global _NC
    if _NC is None:
        _NC = build()
    return _NC


def _make_in_maps(inputs):
    e1 = np.asarray(inputs["embedding1"], dtype=np.float32)
    e2 = np.asarray(inputs["all_embeddings2"], dtype=np.float32)
    w1 = np.asarray(inputs["W1"], dtype=np.float32)
    w2 = np.asarray(inputs["W2"], dtype=np.float32)
    in_maps = []
    for k in range(M):
        sl = slice(k * BL, (k + 1) * BL)
        in_maps.append(
            {
                "embedding1": np.ascontiguousarray(e1[sl]),
                "all_embeddings2": np.ascontiguousarray(e2[:, sl, :]),
                "W1": w1,
                "W2": w2,
            }
        )
    return in_maps


def _unshard(arr):
    return arr.reshape(128, 8, 128).transpose(2, 1, 0).reshape(BL, D)


def _run(inputs, trace=False, **kwargs):
    nc = _get_nc()
    res = run_bass_kernel_spmd(
        nc, _make_in_maps(inputs), core_ids=list(range(M)), trace=trace, **kwargs
    )
    full = np.concatenate(
        [_unshard(res.results[k]["out"]) for k in range(M)], axis=0
    )
    return full, res


def kernel(**inputs):
    full, _ = _run(inputs)
    return full


# revision 21
# speedup vs baseline: 1.0251x; 1.0251x over previous
"""Bass/Tile TRN2 kernel for nn_MessageAggregation.

Computes: s = sum_n e2[n]; out = leaky_relu((e1+s) @ W1.T + (e1*s) @ W2.T)

Sharding: data-parallel over batch B=8192 across 8 NeuronCores (1024 rows
per core); W1/W2 replicated.

Per-core layout: SBUF [128 partitions, 1024 free]; partition p holds batch
rows 8p..8p+7 (4 KB contiguous per partition per DMA descriptor). The
kernel is DMA-bound (~32 MB of all_embeddings2 per core at ~400 GB/s;
stream floor ~85 us), so the shape of the head/tail around the stream is
what matters.

Stream: the n-reduction is split DVE 42 / GpSimd 21 slices. DVE
accumulates in PSUM (1 SBUF read stream), GpSimd in SBUF; when both
engines run 3-stream SBUF ops concurrently with the DMA writes, the whole
machine slows ~30% (measured), so this split is load-bearing.

Slice 63 arrives as 8 per-chunk 64 KB DMAs: final adds stagger per chunk
(DVE chunks 4-7 into the PSUM accumulator right after its last stream
add; GpSimd chunks 0-3 into its SBUF accumulator), so each chunk's fold
starts as soon as its columns are final.

Tail: DVE folds s_dve+s_gps as [128,256] pairs (doubling as the
PSUM->SBUF move), f32r transposes on PE, st quad-copies on scalar, x2t
pair-muls on DVE; per half two 512-wide f32r matmuls accumulate onto the
head e1@W1.T PSUM group (reopened with start=False), lrelu per half on
scalar, stores issued from the otherwise idle sync engine.
Free position f = j*128 + p maps to batch row 8p + j; the host gather
un-permutes with a reshape/transpose (not timed).
"""

import sys

for _p in ("/opt/trn_rl_repo",):
    if _p not in sys.path:
        sys.path.insert(0, _p)

import numpy as np

import concourse.bacc as bacc
import concourse.mybir as mybir
import concourse.tile as tile
from concourse.masks import make_identity
from concourse.bass_utils import run_bass_kernel_spmd

B, N, D = 8192, 64, 128
M = 8  # cores
BL = B // M  # 1024 rows per core
R = BL // 128  # chunks per core (8)
F = BL  # free width of the [128, F] working layout
H = F // 2
F32 = mybir.dt.float32
F32R = mybir.dt.float32r
NEG_SLOPE = 0.01
LRELU = mybir.ActivationFunctionType.Lrelu

# Stream routing for slices 0..62 (slice 63 is chunked): D -> DVE, G -> GpSimd.
PLAN = (
    [(4, "DDGD"), (4, "DGDG")] * 7
    + [(4, "GDDG"), (1, "D"), (1, "D"), (1, "D")]
)
assert sum(g for g, _ in PLAN) == N - 1

# Slice-63 chunk DMAs land DVE's chunks first.
CHUNK_DMA_ORDER = [4, 5, 6, 7, 0, 1, 2, 3]


def build(load_bufs: int = 7):
    nc = bacc.Bacc(
        "TRN2",
        target_bir_lowering=False,
        debug=False,
        enable_asserts=False,
        num_devices=M,
    )
    e1 = nc.dram_tensor("embedding1", [BL, D], F32, kind="ExternalInput").ap()
    e2 = nc.dram_tensor("all_embeddings2", [N, BL, D], F32, kind="ExternalInput").ap()
    w1 = nc.dram_tensor("W1", [D, D], F32, kind="ExternalInput").ap()
    w2 = nc.dram_tensor("W2", [D, D], F32, kind="ExternalInput").ap()
    out = nc.dram_tensor("out", [D, BL], F32, kind="ExternalOutput").ap()

    e1_r = e1.rearrange("(p r) d -> p (r d)", p=128)  # [128, 1024]
    e2_r = e2.rearrange("n (p r) d -> p n (r d)", p=128)  # [128, 64, 1024]

    with tile.TileContext(nc) as tc:
        with (
            tc.tile_pool(name="const", bufs=1) as cpool,
            tc.tile_pool(name="load", bufs=load_bufs) as lpool,
            tc.tile_pool(name="last", bufs=8) as lastpool,
            tc.tile_pool(name="act", bufs=1) as apool,
            tc.tile_pool(name="sdve", bufs=1, space="PSUM") as sdpool,
            tc.tile_pool(name="ops", bufs=1, space="PSUM") as opool,
            tc.tile_pool(name="trps", bufs=4, space="PSUM") as trpool,
        ):
            ident = cpool.tile([128, 128], F32)
            make_identity(nc, ident[:])
            ident_r = cpool.tile([128, 128], F32R)
            nc.scalar.copy(out=ident_r[:], in_=ident[:])

            w1_sb = cpool.tile([128, 128], F32)
            nc.scalar.dma_start(out=w1_sb[:], in_=w1)
            w2_sb = cpool.tile([128, 128], F32)
            nc.scalar.dma_start(out=w2_sb[:], in_=w2)
            e1_sb = apool.tile([128, F], F32)
            nc.scalar.dma_start(out=e1_sb[:], in_=e1_r)

            # W.T in SBUF: stationary operand of the output matmuls. fp32
            # for the exact e1-term at the head; f32r for the single-pass
            # tail matmuls.
            w1t_ps = trpool.tile([128, 128], F32, tag="tr")
            nc.tensor.transpose(w1t_ps[:], w1_sb[:], ident[:])
            w1t = cpool.tile([128, 128], F32)
            nc.scalar.copy(out=w1t[:], in_=w1t_ps[:])
            w1t_r = cpool.tile([128, 128], F32R)
            nc.scalar.copy(out=w1t_r[:], in_=w1t_ps[:])
            w2t_ps = trpool.tile([128, 128], F32, tag="tr")
            nc.tensor.transpose(w2t_ps[:], w2_sb[:], ident[:])
            w2t_r = cpool.tile([128, 128], F32R)
            nc.scalar.copy(out=w2t_r[:], in_=w2t_ps[:])

            # Dummy lrelu at the head so its activation table is resident
            # before the tail (a mid-tail ACT_TABLE_LOAD costs 1.3us on the
            # critical path once scalar COPY ops intervene).
            warm = cpool.tile([128, 8], F32)
            nc.scalar.activation(warm[:], ident[:, 0:8], LRELU, alpha=NEG_SLOPE)

            # e1^T pre-stage: chunk j of e1 transposed -> e1t[:, j*128:(j+1)*128]
            e1t = apool.tile([128, F], F32)
            for j in range(R):
                sl = slice(j * 128, (j + 1) * 128)
                tp = trpool.tile([128, 128], F32, tag="tr")
                nc.tensor.transpose(tp[:], e1_sb[:, sl], ident[:])
                nc.scalar.copy(out=e1t[:, sl], in_=tp[:])

            # e1 @ W1.T term of out_T, as a CLOSED accumulation group per
            # half (PE idle during the stream; tail reopens with start=False).
            o_ps0 = opool.tile([128, H], F32)
            o_ps1 = opool.tile([128, H], F32)
            o_ps = [o_ps0, o_ps1]
            for h in range(2):
                hs = slice(h * H, (h + 1) * H)
                nc.tensor.matmul(
                    o_ps[h][:], lhsT=w1t[:], rhs=e1t[:, hs], start=True, stop=True
                )

            # ---- stream ----
            s_dve = sdpool.tile([128, F], F32)  # PSUM accumulator (2 banks)
            s_gps = apool.tile([128, F], F32)  # GpSimd SBUF accumulator
            seen = {"D": 0, "G": 0}
            base = 0
            for gl, routing in PLAN:
                t = lpool.tile([128, gl * F], F32, tag="load")
                nc.sync.dma_start(
                    out=t[:].rearrange("p (n f) -> p n f", n=gl),
                    in_=e2_r[:, base : base + gl, :],
                )
                for g in range(gl):
                    eng = routing[g]
                    seen[eng] += 1
                    sl = t[:, g * F : (g + 1) * F]
                    if eng == "D":
                        if seen["D"] == 1:
                            nc.vector.tensor_copy(out=s_dve[:], in_=sl)
                        else:
                            nc.vector.tensor_add(out=s_dve[:], in0=s_dve[:], in1=sl)
                    else:
                        if seen["G"] == 1:
                            nc.gpsimd.tensor_copy(out=s_gps[:], in_=sl)
                        else:
                            nc.gpsimd.tensor_add(out=s_gps[:], in0=s_gps[:], in1=sl)
                base += gl

            # Last slice: 8 per-chunk DMAs, staggered final adds.
            last_t = {}
            for c in CHUNK_DMA_ORDER:
                tcch = lastpool.tile([128, 128], F32, tag=f"lc{c}")
                nc.sync.dma_start(
                    out=tcch[:], in_=e2_r[:, N - 1, c * 128 : (c + 1) * 128]
                )
                last_t[c] = tcch
            for c in [4, 5, 6, 7]:
                sl = slice(c * 128, (c + 1) * 128)
                nc.vector.tensor_add(
                    out=s_dve[:, sl], in0=s_dve[:, sl], in1=last_t[c][:]
                )
            for c in [0, 1, 2, 3]:
                sl = slice(c * 128, (c + 1) * 128)
                nc.gpsimd.tensor_add(
                    out=s_gps[:, sl], in0=s_gps[:, sl], in1=last_t[c][:]
                )

            # ---- tail ----
            s_sb = apool.tile([128, F], F32R)
            st = apool.tile([128, F], F32R)
            x2t = apool.tile([128, F], F32R)
            out_sb = apool.tile([128, F], F32)

            # DVE folds (pairs; also the PSUM->SBUF move), then pair-muls.
            tps = {}

            def fold_pair(a):
                sl = slice(a * 128, (a + 2) * 128)
                nc.vector.tensor_add(
                    out=s_sb[:, sl], in0=s_dve[:, sl], in1=s_gps[:, sl]
                )

            def tr_chunk(j):
                sl = slice(j * 128, (j + 1) * 128)
                tp = trpool.tile([128, 128], F32R, tag="tr")
                nc.tensor.transpose(tp[:], s_sb[:, sl], ident_r[:])
                tps[j] = tp

            fold_pair(4)
            fold_pair(6)
            fold_pair(0)
            fold_pair(2)
            for j in [4, 5, 6, 7, 0, 1, 2, 3]:
                tr_chunk(j)

            # x2t muls per chunk (DVE; tp lives in PSUM so DVE only).
            for j in [4, 5, 6, 7, 0, 1, 2, 3]:
                sl = slice(j * 128, (j + 1) * 128)
                nc.vector.tensor_mul(out=x2t[:, sl], in0=e1t[:, sl], in1=tps[j][:])

            # st copies per chunk on scalar (all before the acts).
            for j in [4, 5, 6, 7, 0, 1, 2, 3]:
                sl = slice(j * 128, (j + 1) * 128)
                nc.scalar.copy(out=st[:, sl], in_=tps[j][:])

            for h in (1, 0):
                hs = slice(h * H, (h + 1) * H)
                nc.tensor.matmul(
                    o_ps[h][:], lhsT=w1t_r[:], rhs=st[:, hs], start=False, stop=False
                )
                nc.tensor.matmul(
                    o_ps[h][:], lhsT=w2t_r[:], rhs=x2t[:, hs], start=False, stop=True
                )
                nc.scalar.activation(out_sb[:, hs], o_ps[h][:], LRELU, alpha=NEG_SLOPE)
                nc.sync.dma_start(out=out[:, hs], in_=out_sb[:, hs])

    nc.compile()
    return nc


_NC = None


def _get_nc():
    global _NC
    if _NC is None:
        _NC = build()
    return _NC


def _make_in_maps(inputs):
    e1 = np.asarray(inputs["embedding1"], dtype=np.float32)
    e2 = np.asarray(inputs["all_embeddings2"], dtype=np.float32)
    w1 = np.asarray(inputs["W1"], dtype=np.float32)
    w2 = np.asarray(inputs["W2"], dtype=np.float32)
    in_maps = []
    for k in range(M):
        sl = slice(k * BL, (k + 1) * BL)
        in_maps.append(
            {
                "embedding1": np.ascontiguousarray(e1[sl]),
                "all_embeddings2": np.ascontiguousarray(e2[:, sl, :]),
                "W1": w1,
                "W2": w2,
            }
        )
    return in_maps


def _unshard(arr):
    # arr: out_T [o=128, f=1024] with f = j*128 + p <-> batch row 8p + j
    return arr.reshape(128, 8, 128).transpose(2, 1, 0).reshape(BL, D)


def _run(inputs, trace=False, **kwargs):
    nc = _get_nc()
    res = run_bass_kernel_spmd(
        nc, _make_in_maps(inputs), core_ids=list(range(M)), trace=trace, **kwargs
    )
    full = np.concatenate(
        [_unshard(res.results[k]["out"]) for k in range(M)], axis=0
    )
    return full, res


def kernel(**inputs):
    full, _ = _run(inputs)
    return full
